# revision 25
# baseline (speedup 1.0000x reference)
"""Trainium2 Bass kernel for the spectral-spatial attention layer.

Sharding: 8 NeuronCores = batch(4) x image-half(2). Each core computes its
128-row half of one batch element. Inside a core the half is split into two
64-row sub-blocks packed on SBUF partitions as p = s*64 + c (s in {0,1},
c = channel), so every engine runs at full 128-partition occupancy and the
1x1 convs become 128-contraction block-diagonal matmuls (W (x) I_2).

Algebra: q = Wq@x is never materialized. The channel attention needs only
u = x @ softmax(Wk@x) (exp + fused multiply-reduce), and the spatial map
a = (Wq^T attn_norm)^T x is a 2-row matmul. out_spe folds into the output
matmul as (Wout diag(attn_norm) Wv_spe) @ x. t = mg_w2@m1 + b2 is computed
directly from mask as (mg_w2 mg_w1)@mask + (mg_w2 b1 + b2), removing the
m1 halo and the m1->t serialization. The global-softmax partial sums
(u, S0) are exchanged between half-image pairs with a tiny [128,2]
AllReduce that overlaps the depthwise conv. The 7x7 single-channel conv is
a row-Toeplitz matmul; the 5x5 depthwise conv runs as 10 DoubleRow fp8
diagonal-matmul tap-pairs + 5 single taps on the tensor engine.

DMA: inputs load as 4 full-128-partition ~1.2MB pieces per tensor (2D
partition AP packing both sub-blocks per piece); params load as 3 packed
tensors; the a-map transposes to row-partitions with 2 DMAs; output stores
batch 4 chunks (0.5MB) per DMA.
"""

import os
import numpy as np
import ml_dtypes
import bass_rust
from contextlib import ExitStack

from concourse import bacc
import concourse.mybir as mybir
import concourse.tile as tile
from concourse.bass_utils import run_bass_kernel_spmd

BF16 = mybir.dt.bfloat16
FP8 = mybir.dt.float8e4
F32 = mybir.dt.float32
AF = mybir.ActivationFunctionType
OP = mybir.AluOpType

B, C, H, W = 4, 64, 256, 256
HALF = 128            # rows per core
R = 64                # rows per sub-block
FR = 70               # frame rows per sub-block (own 64 + 3 halo each side)
EXT = 134             # rows in per-core dram input (128 + 3 + 3)
NO = R * W            # 16384 own cols
CH = 512              # chunk cols (2 rows)
OBG = 2               # output chunks batched per DMA

# packed param layouts
PBF_COLS = 5 * 128 + 7 * 64 + 128     # lvspa lm1 ltd lk lout | t7 | ones2
PF8_COLS = 10 * 256 + 5 * 128         # c5 DoubleRow pairs | c5 singles
PF32_COLS = 4 * 64 + 3 + 8            # lupq wq wvspe woutT | b1 b2d bdw | rv b2drv

# tap split for the 5x5 depthwise conv.
# DoubleRow tap pairs (i0, j): covers taps (i0, j) and (i0+1, j).
DR_PAIRS = [(i0, j) for j in range(-2, 3) for i0 in (-2, 0)]

_cache = {}


def _build():
    nc = bacc.Bacc(num_devices=8)

    x_d = nc.declare_dram_parameter("x_ext", [128, FR, W], BF16,
                                    isOutput=False)
    m_d = nc.declare_dram_parameter("mask_ext", [128, FR, W], BF16,
                                    isOutput=False)
    pbf_d = nc.declare_dram_parameter("pbf", [128, PBF_COLS], BF16,
                                      isOutput=False)
    pf8_d = nc.declare_dram_parameter("pf8", [128, PF8_COLS], FP8,
                                      isOutput=False)
    pf32_d = nc.declare_dram_parameter("pf32", [128, PF32_COLS], F32,
                                       isOutput=False)
    out_d = nc.declare_dram_parameter("out", [128, NO], BF16, isOutput=True)

    in_cc = nc.dram_tensor("in_cc", [128, 2], F32)
    out_cc = nc.dram_tensor("out_cc", [128, 2], F32)

    with ExitStack() as ctx:
        tc = ctx.enter_context(tile.TileContext(nc))
        pp = ctx.enter_context(tc.tile_pool(name="persist", bufs=1))
        cp = ctx.enter_context(tc.tile_pool(name="chunks", bufs=3))
        obp = ctx.enter_context(tc.tile_pool(name="obp", bufs=2))
        ps = ctx.enter_context(tc.tile_pool(name="psum", bufs=3, space="PSUM"))
        psd = ctx.enter_context(tc.tile_pool(name="psumd", bufs=2, space="PSUM"))
        ps1 = ctx.enter_context(tc.tile_pool(name="psum1", bufs=1, space="PSUM"))

        # ---- persistent tiles ----
        x_sb = pp.tile([128, FR, W], BF16, tag="x_sb")
        m_sb = pp.tile([128, FR, W], BF16, tag="m_sb")
        m1_sb = pp.tile([128, R, W], BF16, tag="m1_sb")
        t_sb = pp.tile([128, FR, 272], FP8, tag="t_sb")
        pbf = pp.tile([128, PBF_COLS], BF16, tag="pbf")
        pf8 = pp.tile([128, PF8_COLS], FP8, tag="pf8")
        pf32 = pp.tile([128, PF32_COLS], F32, tag="pf32")
        big2 = pp.tile([2, FR * W], BF16, tag="big2")  # a-map stage, then gate
        s_parts = pp.tile([128, 32], F32, tag="s_parts")
        leff = pp.tile([128, 128], BF16, tag="leff")
        la = pp.tile([128, 2], BF16, tag="la")
        fr0 = pp.tile([128, W + 6], BF16, tag="fr0")
        fr1 = pp.tile([128, W + 6], BF16, tag="fr1")
        c7s = pp.tile([64, 2 * W], BF16, tag="c7s")

        # packed param views
        lvspa = pbf[:, 0:128]
        lm1 = pbf[:, 128:256]
        ltd = pbf[:, 256:384]
        lk = pbf[:, 384:512]
        lout = pbf[:, 512:640]
        t7 = pbf[:, 640:640 + 7 * 64]
        ones2 = pbf[0:2, 1088:1216]
        c5p = pf8[:, 0:2560].rearrange("p (n q m) -> p n q m", n=10, q=2)
        c5s = pf8[:, 2560:3200].rearrange("p (n m) -> p n m", n=5)
        lupq = pf32[:, 0:64]
        wq_sb = pf32[0:64, 64:128]
        wvspe = pf32[0:64, 128:192]
        woutT = pf32[0:64, 192:256]
        b1 = pf32[:, 256:257]
        b2d = pf32[:, 257:258]
        bdw = pf32[:, 258:259]
        rv = pf32[:, 259:263]
        b2drv = pf32[:, 263:267]

        # ---- input loads: few big full-128-partition DMAs ----
        nc.scalar.dma_start(pbf[:], pbf_d[:, :])
        nc.scalar.dma_start(pf8[:], pf8_d[:, :])
        nc.scalar.dma_start(pf32[:], pf32_d[:, :])

        pieces = [(r0, min(r0 + 10, FR)) for r0 in range(0, FR, 10)]

        x_flat = x_sb.rearrange("p r w -> p (r w)")
        m_flat = m_sb.rearrange("p r w -> p (r w)")
        m1_flat = m1_sb.rearrange("p r w -> p (r w)")

        # zero w-pads of t
        tv = t_sb
        nc.vector.memset(tv[:, :, 0:8], 0.0)
        nc.vector.memset(tv[:, :, W + 8:272], 0.0)

        # ---- phases 2/1/3 pipelined with the input loads ----
        # phase 2: k = Wk@x, e = exp(k), u/S0 partials (own rows)
        u_acc = pp.tile([128, CH], BF16, tag="u_acc")
        nc.vector.memset(u_acc[:], 0.0)

        def emit_p2(k):
            lo = (3 + 2 * k) * W
            kps = ps.tile([128, CH], F32, tag="mm", name=f"kps{k}")
            nc.tensor.matmul(kps[:], lk, x_flat[:, lo:lo + CH],
                             start=True, stop=True)
            e_ch = cp.tile([128, CH], BF16, tag="e_ch", bufs=2,
                           name=f"e_ch{k}")
            nc.scalar.activation(e_ch[:], kps[:], AF.Exp,
                                 accum_out=s_parts[:, k:k + 1])
            scr = cp.tile([128, CH], BF16, tag="scr", bufs=2, name=f"scr{k}")
            nc.vector.tensor_tensor(scr[:], x_flat[:, lo:lo + CH], e_ch[:],
                                    OP.mult)
            nc.vector.tensor_tensor(u_acc[:], u_acc[:], scr[:], OP.add)

        def emit_p1(k):
            fr = 3 + 2 * k
            mps = ps.tile([128, CH], F32, tag="mm", name=f"mps{k}")
            nc.tensor.matmul(mps[:], lm1, m_flat[:, fr * W:fr * W + CH],
                             start=True, stop=True)
            nc.vector.tensor_scalar_add(m1_flat[:, 2 * k * W:2 * k * W + CH],
                                        mps[:], b1)

        def emit_p3(k):
            tps = ps.tile([128, CH], F32, tag="mm", name=f"tps{k}")
            nc.tensor.matmul(tps[:], ltd,
                             m_flat[:, (1 + 2 * k) * W:(1 + 2 * k) * W + CH],
                             start=True, stop=True)
            if k in (0, 33):
                # edge chunks: validity mask folded as out = in*rv + b2d*rv
                tpv = tps.rearrange("p (r w) -> p r w", w=W)
                for r in range(2):
                    col = r if k == 0 else 2 + r
                    nc.scalar.activation(tv[:, 1 + 2 * k + r, 8:W + 8],
                                         tpv[:, r], AF.Identity,
                                         bias=b2drv[:, col:col + 1],
                                         scale=rv[:, col:col + 1])
            else:
                nc.scalar.activation(tv[:, 1 + 2 * k:3 + 2 * k, 8:W + 8],
                                     tps[:], AF.Identity, bias=b2d)

        p2_done = p1_done = p3_done = 0
        for r0, r1 in pieces:
            nc.sync.dma_start(x_sb[:, r0:r1, :], x_d[:, r0:r1, :])
            nc.scalar.dma_start(m_sb[:, r0:r1, :], m_d[:, r0:r1, :])
            while p2_done < 32 and 4 + 2 * p2_done < r1:
                emit_p2(p2_done)
                p2_done += 1
            while p1_done < 32 and 4 + 2 * p1_done < r1:
                emit_p1(p1_done)
                p1_done += 1
            while p3_done < 34 and 2 + 2 * p3_done < r1:
                emit_p3(p3_done)
                p3_done += 1
        for k in range(p2_done, 32):
            emit_p2(k)
        for k in range(p1_done, 32):
            emit_p1(k)
        for k in range(p3_done, 34):
            emit_p3(k)

        u_red = pp.tile([128, 1], F32, tag="u_red")
        s_red2 = pp.tile([128, 1], F32, tag="s_red2")
        nc.vector.tensor_reduce(u_red[:], u_acc[:], mybir.AxisListType.X,
                                OP.add)
        nc.vector.tensor_reduce(s_red2[:], s_parts[:], mybir.AxisListType.X,
                                OP.add)
        cc_sb = pp.tile([128, 2], F32, tag="cc_sb")
        nc.vector.tensor_copy(cc_sb[:, 0:1], u_red[:])
        nc.vector.tensor_copy(cc_sb[:, 1:2], s_red2[:])
        nc.sync.dma_start(in_cc[:, :], cc_sb[:])
        nc.gpsimd.collective_compute(
            "AllReduce", OP.add,
            replica_groups=[[0, 1], [2, 3], [4, 5], [6, 7]],
            ins=[in_cc.ap()], outs=[out_cc.ap()],
        )
        cc2 = pp.tile([128, 2], F32, tag="cc2")
        nc.sync.dma_start(cc2[:], out_cc[:, :])

        # ---- phase 7 prepass: CC-independent front work (va, conv5,
        # sigmoid) for the first chunks, filling the PE queue across the
        # collective wait.
        N_PRE = 5
        va_pre, am_pre = [], []
        for k in range(N_PRE):
            va_pre.append(pp.tile([128, CH], BF16, tag=f"va_pre{k}",
                                  name=f"va_pre{k}"))
            am_pre.append(pp.tile([128, CH], BF16, tag=f"am_pre{k}",
                                  name=f"am_pre{k}"))

        def emit_front(k, va_t, am_t):
            fr = 3 + 2 * k
            lo = fr * W
            t3 = t_sb
            vps = ps.tile([128, CH], F32, tag="mm", name=f"vps{k}")
            nc.tensor.matmul(vps[:], lvspa, x_flat[:, lo:lo + CH],
                             start=True, stop=True)
            nc.scalar.activation(va_t[:], vps[:], AF.Copy)
            dwps = psd.tile([128, 2, W], F32, tag="dw", name=f"dwps{k}")
            first = True
            for n, (i0, j) in enumerate(DR_PAIRS):
                base = t3[:, fr + i0:fr + i0 + 2, 8 + j:8 + j + W]
                rhs = base.copy()
                _ps = rhs.ap[0][0]
                rhs.ap = bass_rust.VecI64Pair(
                    [(_ps, 128), (272, 2), (272, 2), (1, W)])
                nc.tensor.matmul(dwps[:], c5p[:, n], rhs,
                                 start=first, stop=False,
                                 perf_mode=mybir.MatmulPerfMode.DoubleRow)
                first = False
            for n, j in enumerate(range(-2, 3)):
                nc.tensor.matmul(dwps[:], c5s[:, n],
                                 t3[:, fr + 2:fr + 4, 8 + j:8 + j + W],
                                 start=False, stop=(n == 4))
            nc.scalar.activation(am_t[:], dwps.rearrange("p a b -> p (a b)"),
                                 AF.Sigmoid, bias=bdw)

        for k in range(N_PRE):
            emit_front(k, va_pre[k], am_pre[k])

        # ---- small attention math (f32) ----
        l0ps = ps1.tile([64, 1], F32, tag="small")
        nc.tensor.matmul(l0ps[:], lupq, cc2[:, 0:1], start=True, stop=True)
        l0sb = pp.tile([64, 1], F32, tag="l0sb")
        nc.vector.tensor_copy(l0sb[:], l0ps[:])
        inv64 = pp.tile([128, 1], F32, tag="inv64")
        nc.vector.memset(inv64[:], 1.0 / 64.0)
        s0ps = ps1.tile([1, 1], F32, tag="small")
        nc.tensor.matmul(s0ps[:], inv64[:], cc2[:, 1:2], start=True, stop=True)
        rs0 = pp.tile([1, 1], F32, tag="rs0")
        nc.vector.reciprocal(rs0[:], s0ps[:])
        ones64 = pp.tile([1, 64], F32, tag="ones64")
        nc.vector.memset(ones64[:], 1.0)
        rbps = ps1.tile([64, 1], F32, tag="small")
        nc.tensor.matmul(rbps[:], ones64[:], rs0[:], start=True, stop=True)
        rb = pp.tile([64, 1], F32, tag="rb")
        nc.vector.tensor_copy(rb[:], rbps[:])
        el = pp.tile([64, 1], F32, tag="el")
        nc.scalar.activation(el[:], l0sb[:], AF.Exp, scale=rb[:])
        ones6464 = pp.tile([64, 64], F32, tag="ones6464")
        nc.vector.memset(ones6464[:], 1.0)
        seps = ps1.tile([64, 1], F32, tag="small")
        nc.tensor.matmul(seps[:], ones6464[:], el[:], start=True, stop=True)
        rsum = pp.tile([64, 1], F32, tag="rsum")
        nc.vector.reciprocal(rsum[:], seps[:])
        an = pp.tile([64, 1], F32, tag="an")
        nc.vector.tensor_tensor(an[:], el[:], rsum[:], OP.mult)
        waps = ps1.tile([64, 1], F32, tag="small")
        nc.tensor.matmul(waps[:], wq_sb, an[:], start=True, stop=True)
        wa = pp.tile([64, 1], BF16, tag="wa")
        nc.vector.tensor_copy(wa[:], waps[:])
        nc.vector.memset(la[:], 0.0)
        nc.vector.tensor_copy(la[0:64, 0:1], wa[:])
        nc.vector.tensor_copy(la[64:128, 1:2], wa[:])
        d1 = pp.tile([64, 64], F32, tag="d1")
        nc.vector.tensor_scalar_mul(d1[:], wvspe, an[:])
        eps_ = ps1.tile([64, 64], F32, tag="small")
        nc.tensor.matmul(eps_[:], d1[:], woutT, start=True, stop=True)
        nc.vector.memset(leff[:], 0.0)
        nc.vector.tensor_copy(leff[0:64, 0:64], eps_[:])
        nc.vector.tensor_copy(leff[64:128, 64:128], eps_[:])

        # ---- a-map + 7x7 conv + gate ----
        nc.vector.memset(fr0[:], 0.0)
        nc.vector.memset(fr1[:], 0.0)
        for k in range(35):
            lo = 2 * k * W
            aps_t = ps.tile([2, CH], F32, tag="mm")
            nc.tensor.matmul(aps_t[:], la[:], x_flat[:, lo:lo + CH],
                             start=True, stop=True)
            nc.vector.tensor_copy(big2[:, k * CH:(k + 1) * CH], aps_t[:])
        # scatter the staged a-map to row-partitions: one DMA per frame
        nc.sync.dma_start(fr0[0:FR, 3:3 + W], big2[0:1, :])
        nc.scalar.dma_start(fr1[0:FR, 3:3 + W], big2[1:2, :])
        for s, frame in ((0, fr0), (1, fr1)):
            c7ps = ps.tile([64, W], F32, tag="mm")
            for j in range(7):
                nc.tensor.matmul(c7ps[:], t7[:, j * 64:(j + 1) * 64],
                                 frame[:, j:j + W],
                                 start=(j == 0), stop=(j == 6))
            nc.scalar.activation(c7s[:, s * W:(s + 1) * W], c7ps[:], AF.Sigmoid)
        gate_sb2 = big2  # reuse the stage (WAR dep via tile hazards)
        nc.sync.dma_start(gate_sb2[0:1, 0:NO], c7s[:, 0:W])
        nc.gpsimd.dma_start(gate_sb2[1:2, 0:NO], c7s[:, W:2 * W])

        # ---- phase 7: gate/zz/output per 2-row chunk ----
        ob_cur = None
        for k in range(32):
            fr = 3 + 2 * k
            lo = fr * W
            lo1 = 2 * k * W
            if k < N_PRE:
                va, am = va_pre[k], am_pre[k]
            else:
                va = cp.tile([128, CH], BF16, tag="va", bufs=5)
                am = cp.tile([128, CH], BF16, tag="am", bufs=5)
                emit_front(k, va, am)
            # zz = (gate + m1*(am+1)) * v_spa
            gps_t = ps.tile([128, CH], F32, tag="mm")
            nc.tensor.matmul(gps_t[:], ones2,
                             gate_sb2[:, k * CH:(k + 1) * CH],
                             start=True, stop=True)
            p2 = cp.tile([128, CH], BF16, tag="p2", bufs=3)
            nc.vector.scalar_tensor_tensor(p2[:], va[:], 1.0, gps_t[:],
                                           OP.mult, OP.mult)
            g1 = cp.tile([128, CH], BF16, tag="g1", bufs=3)
            nc.vector.scalar_tensor_tensor(g1[:], am[:], 1.0,
                                           m1_flat[:, lo1:lo1 + CH],
                                           OP.add, OP.mult)
            p1 = cp.tile([128, CH], BF16, tag="p1", bufs=3)
            nc.vector.tensor_tensor(p1[:], g1[:], va[:], OP.mult)
            ptot = cp.tile([128, CH], BF16, tag="ptot", bufs=3)
            nc.vector.tensor_tensor(ptot[:], p1[:], p2[:], OP.add)
            # out = Wout@(p1+p2) + W_eff@x
            ops_t = ps.tile([128, CH], F32, tag="out", bufs=2)
            nc.tensor.matmul(ops_t[:], lout, ptot[:], start=True, stop=False)
            nc.tensor.matmul(ops_t[:], leff[:], x_flat[:, lo:lo + CH],
                             start=False, stop=True)
            if k % OBG == 0:
                ob_cur = obp.tile([128, OBG * CH], BF16, tag="ob")
            nc.scalar.activation(ob_cur[:, (k % OBG) * CH:(k % OBG + 1) * CH],
                                 ops_t[:], AF.Copy)
            if k % OBG == OBG - 1:
                g = k // OBG
                eng = [nc.sync, nc.scalar, nc.gpsimd][g % 3]
                eng.dma_start(out_d[:, (k + 1 - OBG) * CH:(k + 1) * CH],
                              ob_cur[:])

    nc.finalize()
    return nc


def _stage(inputs):
    """Host-side staging: per-core packed bf16 inputs + weight stationaries."""
    f32 = np.float32
    x = np.asarray(inputs["x"], f32)
    mask = np.asarray(inputs["mask"], f32)
    Wq = np.asarray(inputs["Wq"], f32)
    Wk = np.asarray(inputs["Wk"], f32)
    Wv_spe = np.asarray(inputs["Wv_spe"], f32)
    Wv_spa = np.asarray(inputs["Wv_spa"], f32)
    Wup = np.asarray(inputs["Wup"], f32)
    Wout = np.asarray(inputs["Wout"], f32)
    Wnorm = np.asarray(inputs["Wnorm"], f32)
    mg_w1 = np.asarray(inputs["mg_w1"], f32)
    mg_b1 = np.asarray(inputs["mg_b1"], f32)
    mg_w2 = np.asarray(inputs["mg_w2"], f32)
    mg_b2 = np.asarray(inputs["mg_b2"], f32)
    mg_dw = np.asarray(inputs["mg_dw"], f32)
    mg_bdw = np.asarray(inputs["mg_bdw"], f32)

    bf = ml_dtypes.bfloat16
    f8 = ml_dtypes.float8_e4m3

    def blockdiag(w):
        L = np.zeros((128, 128), f32)
        L[0:64, 0:64] = w.T
        L[64:128, 64:128] = w.T
        return L

    l_k = np.zeros((128, 128), f32)
    kb = np.tile(Wk[0][:, None], (1, 64))
    l_k[0:64, 0:64] = kb
    l_k[64:128, 64:128] = kb

    t7 = np.zeros((128, 7 * 64), f32)
    for j in range(7):
        for m in range(64):
            for i3 in range(7):
                kk = m + i3
                if kk < FR:
                    t7[kk, j * 64 + m] = Wnorm[0, 0, i3, j]

    o2 = np.zeros((128, 128), f32)
    o2[0, 0:64] = 1.0
    o2[1, 64:128] = 1.0
    pbf = np.concatenate([
        blockdiag(Wv_spa), blockdiag(mg_w1), blockdiag(mg_w2 @ mg_w1),
        l_k, blockdiag(Wout), t7, o2], axis=1).astype(bf)

    pf8 = np.zeros((128, PF8_COLS), f32)
    for n, (i0, j) in enumerate(DR_PAIRS):
        for q, i in enumerate((i0, i0 + 1)):
            d = np.tile(mg_dw[:, 0, i + 2, j + 2], 2)
            for kk in range(128):
                pf8[kk, n * 256 + q * 128 + kk] = d[kk]
    for n, j in enumerate(range(-2, 3)):
        d = np.tile(mg_dw[:, 0, 4, j + 2], 2)
        for kk in range(128):
            pf8[kk, 2560 + n * 128 + kk] = d[kk]
    pf8 = pf8.astype(f8)

    l_upq = np.zeros((128, 64), f32)
    wupq = (Wup @ Wq).T  # [c, o]
    l_upq[0:64] = wupq
    l_upq[64:128] = wupq

    def pad128(w):
        z = np.zeros((128, 64), f32)
        z[0:64] = w
        return z

    b1_rep = np.tile(mg_b1, 2)[:, None].astype(f32)
    b2d = mg_w2 @ mg_b1 + mg_b2
    b2d_rep = np.tile(b2d, 2)[:, None].astype(f32)
    bdw_rep = np.tile(mg_bdw, 2)[:, None].astype(f32)

    pf32_base = np.concatenate([
        l_upq, pad128(Wq), pad128(Wv_spe), pad128(Wout.T.copy()),
        b1_rep, b2d_rep, bdw_rep], axis=1)

    shared = {"pbf": pbf, "pf8": pf8}

    in_maps = []
    for core in range(8):
        b, h = core // 2, core % 2
        xe = np.zeros((128, FR, W), f32)
        me = np.zeros((128, FR, W), f32)
        for s in range(2):
            r0 = 128 * h + 64 * s - 3
            lo_img, hi_img = max(r0, 0), min(r0 + FR, H)
            xe[s * 64:(s + 1) * 64, lo_img - r0:hi_img - r0] = \
                x[b, :, lo_img:hi_img]
            me[s * 64:(s + 1) * 64, lo_img - r0:hi_img - r0] = \
                mask[b, :, lo_img:hi_img]
        # t row-validity for frame rows {1,2,67,68} of each sub-block
        rvv = np.ones((128, 4), f32)
        for s in range(2):
            for col, frr in enumerate((1, 2, 67, 68)):
                img_row = 128 * h + 64 * s - 3 + frr
                if not (0 <= img_row < H):
                    rvv[s * 64:(s + 1) * 64, col] = 0.0
        m = dict(shared)
        m["x_ext"] = xe.astype(bf)
        m["mask_ext"] = me.astype(bf)
        m["pf32"] = np.concatenate(
            [pf32_base, rvv, b2d_rep * rvv], axis=1).astype(f32)
        in_maps.append(m)
    return in_maps


class _Res:
    def __init__(self, results):
        self.results = results
        self.exec_time_ns = None


def _run_cached(in_maps):
    """Execute the prebuilt NEFF via a cached jitted shard_map callable.

    Mirrors bass2jax.run_bass_via_pjrt but reuses the compiled executable
    across calls (the stock path rebuilds the jit closure every call).
    """
    import jax
    import numpy as np
    from jax.sharding import Mesh, PartitionSpec
    from jax.experimental.shard_map import shard_map
    from concourse import bass2jax, mybir as _mb

    nc = _cache["nc"]
    if "exec" not in _cache:
        bass2jax.install_neuronx_cc_hook()
        in_names, out_names, out_avals, zero_shapes = [], [], [], []
        pname = nc.partition_id_tensor.name if nc.partition_id_tensor else None
        for alloc in nc.m.functions[0].allocations:
            if not isinstance(alloc, _mb.MemoryLocationSet):
                continue
            name = alloc.memorylocations[0].name
            if alloc.kind == "ExternalInput":
                if name != pname:
                    in_names.append(name)
            elif alloc.kind == "ExternalOutput":
                out_names.append(name)
                shape = tuple(alloc.tensor_shape)
                dt = _mb.dt.np(alloc.dtype)
                out_avals.append(jax.core.ShapedArray(shape, dt))
                zero_shapes.append((shape, dt))
        n_params = len(in_names)
        all_names = in_names + out_names
        if pname is not None:
            all_names.append(pname)

        def _body(*args):
            operands = list(args)
            if pname is not None:
                operands.append(bass2jax.partition_id_tensor())
            return tuple(bass2jax._bass_exec_p.bind(
                *operands, out_avals=tuple(out_avals),
                in_names=tuple(all_names), out_names=tuple(out_names),
                lowering_input_output_aliases=(), sim_require_finite=True,
                sim_require_nnan=True, nc=nc))

        devices = jax.devices()[:8]
        mesh = Mesh(np.asarray(devices), ("core",))
        n_outs = len(out_names)
        sharded = jax.jit(
            shard_map(_body, mesh=mesh,
                      in_specs=(PartitionSpec("core"),) * (n_params + n_outs),
                      out_specs=(PartitionSpec("core"),) * n_outs,
                      check_rep=False),
            donate_argnums=tuple(range(n_params, n_params + n_outs)),
            keep_unused=True)
        _cache["exec"] = (sharded, in_names, out_names, zero_shapes, out_avals)
    sharded, in_names, out_names, zero_shapes, out_avals = _cache["exec"]
    concat_in = [np.concatenate([np.asarray(m[n]) for m in in_maps], axis=0)
                 for n in in_names]
    concat_zeros = [np.zeros((8 * s[0], *s[1:]), d) for s, d in zero_shapes]
    outs = sharded(*concat_in, *concat_zeros)
    return _Res([
        {n: np.asarray(outs[i]).reshape(8, *out_avals[i].shape)[c]
         for i, n in enumerate(out_names)}
        for c in range(8)
    ])


def _reset_backend():
    """Drop dead PJRT clients so a retry reconnects to a healthy worker."""
    try:
        import jax._src.xla_bridge as xb
        xb._clear_backends()
    except Exception:
        pass
    _cache.pop("exec", None)


def run(inputs, trace=False):
    if "nc" not in _cache:
        _cache["nc"] = _build()
    in_maps = _stage(inputs)
    last = None
    for attempt in range(2):
        try:
            if not trace:
                try:
                    res = _run_cached(in_maps)
                except Exception:
                    res = run_bass_kernel_spmd(_cache["nc"], in_maps,
                                               core_ids=list(range(8)),
                                               trace=False)
            else:
                res = run_bass_kernel_spmd(_cache["nc"], in_maps,
                                           core_ids=list(range(8)), trace=True)
            break
        except Exception as e:
            last = e
            _reset_backend()
            import time
            time.sleep(2)
    else:
        raise last
    out = np.empty((B, C, H, W), np.float32)
    for core in range(8):
        b, h = core // 2, core % 2
        o = np.asarray(res.results[core]["out"], dtype=np.float32)
        o = o.reshape(2, 64, R, W)  # [s, c, r, w]
        out[b, :, 128 * h:128 * h + 64] = o[0]
        out[b, :, 128 * h + 64:128 * h + 128] = o[1]
    return out, res


def _kernel_subprocess(inputs) -> np.ndarray:
    """Fresh-process fallback: a dead PJRT worker connection poisons the
    whole process, but a new process reconnects to a healthy worker."""
    import subprocess
    import sys
    import tempfile
    import time

    d = tempfile.mkdtemp()
    inp = os.path.join(d, "in.npz")
    outp = os.path.join(d, "out.npy")
    np.savez(inp, **inputs)
    here = os.path.dirname(os.path.abspath(__file__))
    script = (
        "import sys, numpy as np\n"
        f"sys.path.insert(0, {here!r})\n"
        "import kernel\n"
        f"z = np.load({inp!r})\n"
        "out, _ = kernel.run({k: z[k] for k in z.files}, trace=False)\n"
        f"np.save({outp!r}, out)\n"
    )
    last = b""
    for attempt in range(5):
        try:
            r = subprocess.run([sys.executable, "-c", script],
                               capture_output=True, timeout=1200)
            if r.returncode == 0 and os.path.exists(outp):
                return np.load(outp)
            last = r.stderr[-2000:]
        except subprocess.TimeoutExpired:
            last = b"timeout"
        time.sleep(5 + 10 * attempt)
    raise RuntimeError(f"subprocess kernel failed: {last!r}")


def kernel(**inputs) -> np.ndarray:
    try:
        out, _ = run(inputs, trace=False)
        return out
    except Exception:
        return _kernel_subprocess(inputs)


# revision 26
# speedup vs baseline: 1.0791x; 1.0791x over previous
"""Trainium2 Bass kernel for the spectral-spatial attention layer.

Sharding: 8 NeuronCores = batch(4) x image-half(2). Each core computes its
128-row half of one batch element. Inside a core the half is split into two
64-row sub-blocks packed on SBUF partitions as p = s*64 + c (s in {0,1},
c = channel), so every engine runs at full 128-partition occupancy and the
1x1 convs become 128-contraction block-diagonal matmuls (W (x) I_2).

Algebra: q = Wq@x is never materialized. The channel attention needs only
u = x @ softmax(Wk@x) (exp + fused multiply-reduce), and the spatial map
a = (Wq^T attn_norm)^T x is a 2-row matmul. out_spe folds into the output
matmul as (Wout diag(attn_norm) Wv_spe) @ x. t = mg_w2@m1 + b2 is computed
directly from mask as (mg_w2 mg_w1)@mask + (mg_w2 b1 + b2), removing the
m1 halo and the m1->t serialization. The global-softmax partial sums
(u, S0) are exchanged between half-image pairs with a tiny [128,2]
AllReduce that overlaps the depthwise conv. The 7x7 single-channel conv is
a row-Toeplitz matmul; the 5x5 depthwise conv runs as 10 DoubleRow fp8
diagonal-matmul tap-pairs + 5 single taps on the tensor engine.

DMA: inputs load as 4 full-128-partition ~1.2MB pieces per tensor (2D
partition AP packing both sub-blocks per piece); params load as 3 packed
tensors; the a-map transposes to row-partitions with 2 DMAs; output stores
batch 4 chunks (0.5MB) per DMA.
"""

import os
import numpy as np
import ml_dtypes
import bass_rust
from contextlib import ExitStack

from concourse import bacc
import concourse.mybir as mybir
import concourse.tile as tile
from concourse.bass_utils import run_bass_kernel_spmd

BF16 = mybir.dt.bfloat16
FP8 = mybir.dt.float8e4
F32 = mybir.dt.float32
AF = mybir.ActivationFunctionType
OP = mybir.AluOpType

B, C, H, W = 4, 64, 256, 256
HALF = 128            # rows per core
R = 64                # rows per sub-block
FR = 70               # frame rows per sub-block (own 64 + 3 halo each side)
EXT = 134             # rows in per-core dram input (128 + 3 + 3)
NO = R * W            # 16384 own cols
CH = 512              # chunk cols (2 rows)
OBG = 2               # output chunks batched per DMA

# packed param layouts
PBF_COLS = 5 * 128 + 7 * 64 + 128     # lvspa lm1 ltd lk lout | t7 | ones2
PF8_COLS = 10 * 256 + 5 * 128         # c5 DoubleRow pairs | c5 singles
PF32_COLS = 4 * 64 + 3 + 8            # lupq wq wvspe woutT | b1 b2d bdw | rv b2drv

# tap split for the 5x5 depthwise conv.
# DoubleRow tap pairs (i0, j): covers taps (i0, j) and (i0+1, j).
DR_PAIRS = [(i0, j) for j in range(-2, 3) for i0 in (-2, 0)]

_cache = {}


def _build():
    nc = bacc.Bacc(num_devices=8)

    x_d = nc.declare_dram_parameter("x_ext", [128, FR, W], BF16,
                                    isOutput=False)
    m_d = nc.declare_dram_parameter("mask_ext", [128, FR, W], BF16,
                                    isOutput=False)
    pbf_d = nc.declare_dram_parameter("pbf", [128, PBF_COLS], BF16,
                                      isOutput=False)
    pf8_d = nc.declare_dram_parameter("pf8", [128, PF8_COLS], FP8,
                                      isOutput=False)
    pf32_d = nc.declare_dram_parameter("pf32", [128, PF32_COLS], F32,
                                       isOutput=False)
    out_d = nc.declare_dram_parameter("out", [128, NO], BF16, isOutput=True)

    in_cc = nc.dram_tensor("in_cc", [128, 2], F32)
    out_cc = nc.dram_tensor("out_cc", [128, 2], F32)

    with ExitStack() as ctx:
        tc = ctx.enter_context(tile.TileContext(nc))
        pp = ctx.enter_context(tc.tile_pool(name="persist", bufs=1))
        cp = ctx.enter_context(tc.tile_pool(name="chunks", bufs=3))
        obp = ctx.enter_context(tc.tile_pool(name="obp", bufs=2))
        ps = ctx.enter_context(tc.tile_pool(name="psum", bufs=3, space="PSUM"))
        psd = ctx.enter_context(tc.tile_pool(name="psumd", bufs=2, space="PSUM"))
        ps1 = ctx.enter_context(tc.tile_pool(name="psum1", bufs=1, space="PSUM"))

        # ---- persistent tiles ----
        x_sb = pp.tile([128, FR, W], BF16, tag="x_sb")
        m_sb = pp.tile([128, FR, W], BF16, tag="m_sb")
        m1_sb = pp.tile([128, R, W], BF16, tag="m1_sb")
        t_sb = pp.tile([128, FR, 272], FP8, tag="t_sb")
        pbf = pp.tile([128, PBF_COLS], BF16, tag="pbf")
        pf8 = pp.tile([128, PF8_COLS], FP8, tag="pf8")
        pf32 = pp.tile([128, PF32_COLS], F32, tag="pf32")
        big2 = pp.tile([2, FR * W], BF16, tag="big2")  # a-map stage, then gate
        s_parts = pp.tile([128, 32], F32, tag="s_parts")
        leff = pp.tile([128, 128], BF16, tag="leff")
        la = pp.tile([128, 2], BF16, tag="la")
        fr0 = pp.tile([128, W + 6], BF16, tag="fr0")
        fr1 = pp.tile([128, W + 6], BF16, tag="fr1")
        c7s = pp.tile([64, 2 * W], BF16, tag="c7s")

        # packed param views
        lvspa = pbf[:, 0:128]
        lm1 = pbf[:, 128:256]
        ltd = pbf[:, 256:384]
        lk = pbf[:, 384:512]
        lout = pbf[:, 512:640]
        t7 = pbf[:, 640:640 + 7 * 64]
        ones2 = pbf[0:2, 1088:1216]
        c5p = pf8[:, 0:2560].rearrange("p (n q m) -> p n q m", n=10, q=2)
        c5s = pf8[:, 2560:3200].rearrange("p (n m) -> p n m", n=5)
        lupq = pf32[:, 0:64]
        wq_sb = pf32[0:64, 64:128]
        wvspe = pf32[0:64, 128:192]
        woutT = pf32[0:64, 192:256]
        b1 = pf32[:, 256:257]
        b2d = pf32[:, 257:258]
        bdw = pf32[:, 258:259]
        rv = pf32[:, 259:263]
        b2drv = pf32[:, 263:267]

        # ---- input loads: few big full-128-partition DMAs ----
        nc.scalar.dma_start(pbf[:], pbf_d[:, :])
        nc.scalar.dma_start(pf8[:], pf8_d[:, :])
        nc.scalar.dma_start(pf32[:], pf32_d[:, :])

        pieces = [(r0, min(r0 + 10, FR)) for r0 in range(0, FR, 10)]

        x_flat = x_sb.rearrange("p r w -> p (r w)")
        m_flat = m_sb.rearrange("p r w -> p (r w)")
        m1_flat = m1_sb.rearrange("p r w -> p (r w)")

        # zero w-pads of t
        tv = t_sb
        nc.gpsimd.memset(tv[:, :, 0:8], 0.0)
        nc.gpsimd.memset(tv[:, :, W + 8:272], 0.0)

        # ---- phases 2/1/3 pipelined with the input loads ----
        # phase 2: k = Wk@x, e = exp(k), u/S0 partials (own rows)
        u_acc = pp.tile([128, CH], BF16, tag="u_acc")
        nc.vector.memset(u_acc[:], 0.0)

        def emit_p2(k):
            lo = (3 + 2 * k) * W
            kps = ps.tile([128, CH], F32, tag="mm", name=f"kps{k}")
            nc.tensor.matmul(kps[:], lk, x_flat[:, lo:lo + CH],
                             start=True, stop=True)
            e_ch = cp.tile([128, CH], BF16, tag="e_ch", bufs=2,
                           name=f"e_ch{k}")
            nc.scalar.activation(e_ch[:], kps[:], AF.Exp,
                                 accum_out=s_parts[:, k:k + 1])
            scr = cp.tile([128, CH], BF16, tag="scr", bufs=2, name=f"scr{k}")
            nc.vector.tensor_tensor(scr[:], x_flat[:, lo:lo + CH], e_ch[:],
                                    OP.mult)
            nc.vector.tensor_tensor(u_acc[:], u_acc[:], scr[:], OP.add)

        def emit_p1(k):
            fr = 3 + 2 * k
            mps = ps.tile([128, CH], F32, tag="mm", name=f"mps{k}")
            nc.tensor.matmul(mps[:], lm1, m_flat[:, fr * W:fr * W + CH],
                             start=True, stop=True)
            nc.vector.tensor_scalar_add(m1_flat[:, 2 * k * W:2 * k * W + CH],
                                        mps[:], b1)

        def emit_p3(k):
            tps = ps.tile([128, CH], F32, tag="mm", name=f"tps{k}")
            nc.tensor.matmul(tps[:], ltd,
                             m_flat[:, (1 + 2 * k) * W:(1 + 2 * k) * W + CH],
                             start=True, stop=True)
            if k in (0, 33):
                # edge chunks: validity mask folded as out = in*rv + b2d*rv
                tpv = tps.rearrange("p (r w) -> p r w", w=W)
                for r in range(2):
                    col = r if k == 0 else 2 + r
                    nc.scalar.activation(tv[:, 1 + 2 * k + r, 8:W + 8],
                                         tpv[:, r], AF.Identity,
                                         bias=b2drv[:, col:col + 1],
                                         scale=rv[:, col:col + 1])
            else:
                nc.scalar.activation(tv[:, 1 + 2 * k:3 + 2 * k, 8:W + 8],
                                     tps[:], AF.Identity, bias=b2d)

        p2_done = 0
        for r0, r1 in pieces:
            nc.sync.dma_start(x_sb[:, r0:r1, :], x_d[:, r0:r1, :])
            nc.scalar.dma_start(m_sb[:, r0:r1, :], m_d[:, r0:r1, :])
            while p2_done < 32 and 4 + 2 * p2_done < r1:
                emit_p2(p2_done)
                p2_done += 1
        for k in range(p2_done, 32):
            emit_p2(k)

        u_red = pp.tile([128, 1], F32, tag="u_red")
        s_red2 = pp.tile([128, 1], F32, tag="s_red2")
        nc.vector.tensor_reduce(u_red[:], u_acc[:], mybir.AxisListType.X,
                                OP.add)
        nc.vector.tensor_reduce(s_red2[:], s_parts[:], mybir.AxisListType.X,
                                OP.add)
        cc_sb = pp.tile([128, 2], F32, tag="cc_sb")
        nc.vector.tensor_copy(cc_sb[:, 0:1], u_red[:])
        nc.vector.tensor_copy(cc_sb[:, 1:2], s_red2[:])
        nc.sync.dma_start(in_cc[:, :], cc_sb[:])
        nc.gpsimd.collective_compute(
            "AllReduce", OP.add,
            replica_groups=[[0, 1], [2, 3], [4, 5], [6, 7]],
            ins=[in_cc.ap()], outs=[out_cc.ap()],
        )
        cc2 = pp.tile([128, 2], F32, tag="cc2")
        nc.sync.dma_start(cc2[:], out_cc[:, :])

        # ---- phase 3: t chunks (feed the conv fronts across the cc wait)
        for k in range(34):
            emit_p3(k)

        # ---- phase 7 prepass: CC-independent front work (va, conv5,
        # sigmoid) for the first chunks, filling the PE queue across the
        # collective wait.
        N_PRE = 5
        va_pre, am_pre = [], []
        for k in range(N_PRE):
            va_pre.append(pp.tile([128, CH], BF16, tag=f"va_pre{k}",
                                  name=f"va_pre{k}"))
            am_pre.append(pp.tile([128, CH], BF16, tag=f"am_pre{k}",
                                  name=f"am_pre{k}"))

        def emit_front(k, va_t, am_t):
            fr = 3 + 2 * k
            lo = fr * W
            t3 = t_sb
            vps = ps.tile([128, CH], F32, tag="mm", name=f"vps{k}")
            nc.tensor.matmul(vps[:], lvspa, x_flat[:, lo:lo + CH],
                             start=True, stop=True)
            nc.scalar.activation(va_t[:], vps[:], AF.Copy)
            dwps = psd.tile([128, 2, W], F32, tag="dw", name=f"dwps{k}")
            first = True
            for n, (i0, j) in enumerate(DR_PAIRS):
                base = t3[:, fr + i0:fr + i0 + 2, 8 + j:8 + j + W]
                rhs = base.copy()
                _ps = rhs.ap[0][0]
                rhs.ap = bass_rust.VecI64Pair(
                    [(_ps, 128), (272, 2), (272, 2), (1, W)])
                nc.tensor.matmul(dwps[:], c5p[:, n], rhs,
                                 start=first, stop=False,
                                 perf_mode=mybir.MatmulPerfMode.DoubleRow)
                first = False
            for n, j in enumerate(range(-2, 3)):
                nc.tensor.matmul(dwps[:], c5s[:, n],
                                 t3[:, fr + 2:fr + 4, 8 + j:8 + j + W],
                                 start=False, stop=(n == 4))
            nc.scalar.activation(am_t[:], dwps.rearrange("p a b -> p (a b)"),
                                 AF.Sigmoid, bias=bdw)

        for k in range(N_PRE):
            emit_front(k, va_pre[k], am_pre[k])

        # ---- small attention math (f32) ----
        l0ps = ps1.tile([64, 1], F32, tag="small")
        nc.tensor.matmul(l0ps[:], lupq, cc2[:, 0:1], start=True, stop=True)
        l0sb = pp.tile([64, 1], F32, tag="l0sb")
        nc.vector.tensor_copy(l0sb[:], l0ps[:])
        inv64 = pp.tile([128, 1], F32, tag="inv64")
        nc.vector.memset(inv64[:], 1.0 / 64.0)
        s0ps = ps1.tile([1, 1], F32, tag="small")
        nc.tensor.matmul(s0ps[:], inv64[:], cc2[:, 1:2], start=True, stop=True)
        rs0 = pp.tile([1, 1], F32, tag="rs0")
        nc.vector.reciprocal(rs0[:], s0ps[:])
        ones64 = pp.tile([1, 64], F32, tag="ones64")
        nc.vector.memset(ones64[:], 1.0)
        rbps = ps1.tile([64, 1], F32, tag="small")
        nc.tensor.matmul(rbps[:], ones64[:], rs0[:], start=True, stop=True)
        rb = pp.tile([64, 1], F32, tag="rb")
        nc.vector.tensor_copy(rb[:], rbps[:])
        el = pp.tile([64, 1], F32, tag="el")
        nc.scalar.activation(el[:], l0sb[:], AF.Exp, scale=rb[:])
        ones6464 = pp.tile([64, 64], F32, tag="ones6464")
        nc.vector.memset(ones6464[:], 1.0)
        seps = ps1.tile([64, 1], F32, tag="small")
        nc.tensor.matmul(seps[:], ones6464[:], el[:], start=True, stop=True)
        rsum = pp.tile([64, 1], F32, tag="rsum")
        nc.vector.reciprocal(rsum[:], seps[:])
        an = pp.tile([64, 1], F32, tag="an")
        nc.vector.tensor_tensor(an[:], el[:], rsum[:], OP.mult)
        waps = ps1.tile([64, 1], F32, tag="small")
        nc.tensor.matmul(waps[:], wq_sb, an[:], start=True, stop=True)
        wa = pp.tile([64, 1], BF16, tag="wa")
        nc.vector.tensor_copy(wa[:], waps[:])
        nc.vector.memset(la[:], 0.0)
        nc.vector.tensor_copy(la[0:64, 0:1], wa[:])
        nc.vector.tensor_copy(la[64:128, 1:2], wa[:])
        d1 = pp.tile([64, 64], F32, tag="d1")
        nc.vector.tensor_scalar_mul(d1[:], wvspe, an[:])
        eps_ = ps1.tile([64, 64], F32, tag="small")
        nc.tensor.matmul(eps_[:], d1[:], woutT, start=True, stop=True)
        nc.vector.memset(leff[:], 0.0)
        nc.vector.tensor_copy(leff[0:64, 0:64], eps_[:])
        nc.vector.tensor_copy(leff[64:128, 64:128], eps_[:])

        # ---- a-map + 7x7 conv + gate ----
        nc.gpsimd.memset(fr0[:], 0.0)
        nc.gpsimd.memset(fr1[:], 0.0)
        for k in range(35):
            lo = 2 * k * W
            aps_t = ps.tile([2, CH], F32, tag="mm")
            nc.tensor.matmul(aps_t[:], la[:], x_flat[:, lo:lo + CH],
                             start=True, stop=True)
            nc.vector.tensor_copy(big2[:, k * CH:(k + 1) * CH], aps_t[:])
        # scatter the staged a-map to row-partitions: one DMA per frame
        nc.sync.dma_start(fr0[0:FR, 3:3 + W], big2[0:1, :])
        nc.scalar.dma_start(fr1[0:FR, 3:3 + W], big2[1:2, :])
        for s, frame in ((0, fr0), (1, fr1)):
            c7ps = ps.tile([64, W], F32, tag="mm")
            for j in range(7):
                nc.tensor.matmul(c7ps[:], t7[:, j * 64:(j + 1) * 64],
                                 frame[:, j:j + W],
                                 start=(j == 0), stop=(j == 6))
            nc.scalar.activation(c7s[:, s * W:(s + 1) * W], c7ps[:], AF.Sigmoid)
        gate_sb2 = big2  # reuse the stage (WAR dep via tile hazards)
        nc.sync.dma_start(gate_sb2[0:1, 0:NO], c7s[:, 0:W])
        nc.gpsimd.dma_start(gate_sb2[1:2, 0:NO], c7s[:, W:2 * W])

        # ---- phase 1: m1 = mg_w1@mask + b1 (needed from phase 7 on) ----
        for k in range(32):
            emit_p1(k)

        # ---- phase 7: gate/zz/output per 2-row chunk ----
        ob_cur = None
        for k in range(32):
            fr = 3 + 2 * k
            lo = fr * W
            lo1 = 2 * k * W
            if k < N_PRE:
                va, am = va_pre[k], am_pre[k]
            else:
                va = cp.tile([128, CH], BF16, tag="va", bufs=5)
                am = cp.tile([128, CH], BF16, tag="am", bufs=5)
                emit_front(k, va, am)
            # zz = (gate + m1*(am+1)) * v_spa
            gps_t = ps.tile([128, CH], F32, tag="mm")
            nc.tensor.matmul(gps_t[:], ones2,
                             gate_sb2[:, k * CH:(k + 1) * CH],
                             start=True, stop=True)
            p2 = cp.tile([128, CH], BF16, tag="p2", bufs=3)
            nc.vector.scalar_tensor_tensor(p2[:], va[:], 1.0, gps_t[:],
                                           OP.mult, OP.mult)
            g1 = cp.tile([128, CH], BF16, tag="g1", bufs=3)
            nc.vector.scalar_tensor_tensor(g1[:], am[:], 1.0,
                                           m1_flat[:, lo1:lo1 + CH],
                                           OP.add, OP.mult)
            p1 = cp.tile([128, CH], BF16, tag="p1", bufs=3)
            nc.vector.tensor_tensor(p1[:], g1[:], va[:], OP.mult)
            ptot = cp.tile([128, CH], BF16, tag="ptot", bufs=3)
            nc.vector.tensor_tensor(ptot[:], p1[:], p2[:], OP.add)
            # out = Wout@(p1+p2) + W_eff@x
            ops_t = ps.tile([128, CH], F32, tag="out", bufs=2)
            nc.tensor.matmul(ops_t[:], lout, ptot[:], start=True, stop=False)
            nc.tensor.matmul(ops_t[:], leff[:], x_flat[:, lo:lo + CH],
                             start=False, stop=True)
            if k % OBG == 0:
                ob_cur = obp.tile([128, OBG * CH], BF16, tag="ob")
            nc.scalar.activation(ob_cur[:, (k % OBG) * CH:(k % OBG + 1) * CH],
                                 ops_t[:], AF.Copy)
            if k % OBG == OBG - 1:
                g = k // OBG
                eng = [nc.sync, nc.scalar, nc.gpsimd][g % 3]
                eng.dma_start(out_d[:, (k + 1 - OBG) * CH:(k + 1) * CH],
                              ob_cur[:])

    nc.finalize()
    return nc


def _stage(inputs):
    """Host-side staging: per-core packed bf16 inputs + weight stationaries."""
    f32 = np.float32
    x = np.asarray(inputs["x"], f32)
    mask = np.asarray(inputs["mask"], f32)
    Wq = np.asarray(inputs["Wq"], f32)
    Wk = np.asarray(inputs["Wk"], f32)
    Wv_spe = np.asarray(inputs["Wv_spe"], f32)
    Wv_spa = np.asarray(inputs["Wv_spa"], f32)
    Wup = np.asarray(inputs["Wup"], f32)
    Wout = np.asarray(inputs["Wout"], f32)
    Wnorm = np.asarray(inputs["Wnorm"], f32)
    mg_w1 = np.asarray(inputs["mg_w1"], f32)
    mg_b1 = np.asarray(inputs["mg_b1"], f32)
    mg_w2 = np.asarray(inputs["mg_w2"], f32)
    mg_b2 = np.asarray(inputs["mg_b2"], f32)
    mg_dw = np.asarray(inputs["mg_dw"], f32)
    mg_bdw = np.asarray(inputs["mg_bdw"], f32)

    bf = ml_dtypes.bfloat16
    f8 = ml_dtypes.float8_e4m3

    def blockdiag(w):
        L = np.zeros((128, 128), f32)
        L[0:64, 0:64] = w.T
        L[64:128, 64:128] = w.T
        return L

    l_k = np.zeros((128, 128), f32)
    kb = np.tile(Wk[0][:, None], (1, 64))
    l_k[0:64, 0:64] = kb
    l_k[64:128, 64:128] = kb

    t7 = np.zeros((128, 7 * 64), f32)
    for j in range(7):
        for m in range(64):
            for i3 in range(7):
                kk = m + i3
                if kk < FR:
                    t7[kk, j * 64 + m] = Wnorm[0, 0, i3, j]

    o2 = np.zeros((128, 128), f32)
    o2[0, 0:64] = 1.0
    o2[1, 64:128] = 1.0
    pbf = np.concatenate([
        blockdiag(Wv_spa), blockdiag(mg_w1), blockdiag(mg_w2 @ mg_w1),
        l_k, blockdiag(Wout), t7, o2], axis=1).astype(bf)

    pf8 = np.zeros((128, PF8_COLS), f32)
    for n, (i0, j) in enumerate(DR_PAIRS):
        for q, i in enumerate((i0, i0 + 1)):
            d = np.tile(mg_dw[:, 0, i + 2, j + 2], 2)
            for kk in range(128):
                pf8[kk, n * 256 + q * 128 + kk] = d[kk]
    for n, j in enumerate(range(-2, 3)):
        d = np.tile(mg_dw[:, 0, 4, j + 2], 2)
        for kk in range(128):
            pf8[kk, 2560 + n * 128 + kk] = d[kk]
    pf8 = pf8.astype(f8)

    l_upq = np.zeros((128, 64), f32)
    wupq = (Wup @ Wq).T  # [c, o]
    l_upq[0:64] = wupq
    l_upq[64:128] = wupq

    def pad128(w):
        z = np.zeros((128, 64), f32)
        z[0:64] = w
        return z

    b1_rep = np.tile(mg_b1, 2)[:, None].astype(f32)
    b2d = mg_w2 @ mg_b1 + mg_b2
    b2d_rep = np.tile(b2d, 2)[:, None].astype(f32)
    bdw_rep = np.tile(mg_bdw, 2)[:, None].astype(f32)

    pf32_base = np.concatenate([
        l_upq, pad128(Wq), pad128(Wv_spe), pad128(Wout.T.copy()),
        b1_rep, b2d_rep, bdw_rep], axis=1)

    shared = {"pbf": pbf, "pf8": pf8}

    in_maps = []
    for core in range(8):
        b, h = core // 2, core % 2
        xe = np.zeros((128, FR, W), f32)
        me = np.zeros((128, FR, W), f32)
        for s in range(2):
            r0 = 128 * h + 64 * s - 3
            lo_img, hi_img = max(r0, 0), min(r0 + FR, H)
            xe[s * 64:(s + 1) * 64, lo_img - r0:hi_img - r0] = \
                x[b, :, lo_img:hi_img]
            me[s * 64:(s + 1) * 64, lo_img - r0:hi_img - r0] = \
                mask[b, :, lo_img:hi_img]
        # t row-validity for frame rows {1,2,67,68} of each sub-block
        rvv = np.ones((128, 4), f32)
        for s in range(2):
            for col, frr in enumerate((1, 2, 67, 68)):
                img_row = 128 * h + 64 * s - 3 + frr
                if not (0 <= img_row < H):
                    rvv[s * 64:(s + 1) * 64, col] = 0.0
        m = dict(shared)
        m["x_ext"] = xe.astype(bf)
        m["mask_ext"] = me.astype(bf)
        m["pf32"] = np.concatenate(
            [pf32_base, rvv, b2d_rep * rvv], axis=1).astype(f32)
        in_maps.append(m)
    return in_maps


class _Res:
    def __init__(self, results):
        self.results = results
        self.exec_time_ns = None


def _run_cached(in_maps):
    """Execute the prebuilt NEFF via a cached jitted shard_map callable.

    Mirrors bass2jax.run_bass_via_pjrt but reuses the compiled executable
    across calls (the stock path rebuilds the jit closure every call).
    """
    import jax
    import numpy as np
    from jax.sharding import Mesh, PartitionSpec
    from jax.experimental.shard_map import shard_map
    from concourse import bass2jax, mybir as _mb

    nc = _cache["nc"]
    if "exec" not in _cache:
        bass2jax.install_neuronx_cc_hook()
        in_names, out_names, out_avals, zero_shapes = [], [], [], []
        pname = nc.partition_id_tensor.name if nc.partition_id_tensor else None
        for alloc in nc.m.functions[0].allocations:
            if not isinstance(alloc, _mb.MemoryLocationSet):
                continue
            name = alloc.memorylocations[0].name
            if alloc.kind == "ExternalInput":
                if name != pname:
                    in_names.append(name)
            elif alloc.kind == "ExternalOutput":
                out_names.append(name)
                shape = tuple(alloc.tensor_shape)
                dt = _mb.dt.np(alloc.dtype)
                out_avals.append(jax.core.ShapedArray(shape, dt))
                zero_shapes.append((shape, dt))
        n_params = len(in_names)
        all_names = in_names + out_names
        if pname is not None:
            all_names.append(pname)

        def _body(*args):
            operands = list(args)
            if pname is not None:
                operands.append(bass2jax.partition_id_tensor())
            return tuple(bass2jax._bass_exec_p.bind(
                *operands, out_avals=tuple(out_avals),
                in_names=tuple(all_names), out_names=tuple(out_names),
                lowering_input_output_aliases=(), sim_require_finite=True,
                sim_require_nnan=True, nc=nc))

        devices = jax.devices()[:8]
        mesh = Mesh(np.asarray(devices), ("core",))
        n_outs = len(out_names)
        sharded = jax.jit(
            shard_map(_body, mesh=mesh,
                      in_specs=(PartitionSpec("core"),) * (n_params + n_outs),
                      out_specs=(PartitionSpec("core"),) * n_outs,
                      check_rep=False),
            donate_argnums=tuple(range(n_params, n_params + n_outs)),
            keep_unused=True)
        _cache["exec"] = (sharded, in_names, out_names, zero_shapes, out_avals)
    sharded, in_names, out_names, zero_shapes, out_avals = _cache["exec"]
    concat_in = [np.concatenate([np.asarray(m[n]) for m in in_maps], axis=0)
                 for n in in_names]
    concat_zeros = [np.zeros((8 * s[0], *s[1:]), d) for s, d in zero_shapes]
    outs = sharded(*concat_in, *concat_zeros)
    return _Res([
        {n: np.asarray(outs[i]).reshape(8, *out_avals[i].shape)[c]
         for i, n in enumerate(out_names)}
        for c in range(8)
    ])


def _reset_backend():
    """Drop dead PJRT clients so a retry reconnects to a healthy worker."""
    try:
        import jax._src.xla_bridge as xb
        xb._clear_backends()
    except Exception:
        pass
    _cache.pop("exec", None)


def run(inputs, trace=False):
    if "nc" not in _cache:
        _cache["nc"] = _build()
    in_maps = _stage(inputs)
    last = None
    for attempt in range(2):
        try:
            if not trace:
                try:
                    res = _run_cached(in_maps)
                except Exception:
                    res = run_bass_kernel_spmd(_cache["nc"], in_maps,
                                               core_ids=list(range(8)),
                                               trace=False)
            else:
                res = run_bass_kernel_spmd(_cache["nc"], in_maps,
                                           core_ids=list(range(8)), trace=True)
            break
        except Exception as e:
            last = e
            _reset_backend()
            import time
            time.sleep(2)
    else:
        raise last
    out = np.empty((B, C, H, W), np.float32)
    for core in range(8):
        b, h = core // 2, core % 2
        o = np.asarray(res.results[core]["out"], dtype=np.float32)
        o = o.reshape(2, 64, R, W)  # [s, c, r, w]
        out[b, :, 128 * h:128 * h + 64] = o[0]
        out[b, :, 128 * h + 64:128 * h + 128] = o[1]
    return out, res


def _kernel_subprocess(inputs) -> np.ndarray:
    """Fresh-process fallback: a dead PJRT worker connection poisons the
    whole process, but a new process reconnects to a healthy worker."""
    import subprocess
    import sys
    import tempfile
    import time

    d = tempfile.mkdtemp()
    inp = os.path.join(d, "in.npz")
    outp = os.path.join(d, "out.npy")
    np.savez(inp, **inputs)
    here = os.path.dirname(os.path.abspath(__file__))
    script = (
        "import sys, numpy as np\n"
        f"sys.path.insert(0, {here!r})\n"
        "import kernel\n"
        f"z = np.load({inp!r})\n"
        "out, _ = kernel.run({k: z[k] for k in z.files}, trace=False)\n"
        f"np.save({outp!r}, out)\n"
    )
    last = b""
    for attempt in range(5):
        try:
            r = subprocess.run([sys.executable, "-c", script],
                               capture_output=True, timeout=1200)
            if r.returncode == 0 and os.path.exists(outp):
                return np.load(outp)
            last = r.stderr[-2000:]
        except subprocess.TimeoutExpired:
            last = b"timeout"
        time.sleep(5 + 10 * attempt)
    raise RuntimeError(f"subprocess kernel failed: {last!r}")


def kernel(**inputs) -> np.ndarray:
    try:
        out, _ = run(inputs, trace=False)
        return out
    except Exception:
        return _kernel_subprocess(inputs)
